# revision 13
# baseline (speedup 1.0000x reference)
"""Inverse 2D Haar DWT (idwt2) Trainium2 Bass kernel.

Full inputs: approximation/detail_h/detail_v/detail_d each [8, 64, 128, 128] f32.
Full output: [8, 64, 256, 256] f32 with out 2x2 blocks:
  x00 = (a + v + h + d)/2   at [2i,   2j]
  x01 = (a - v + h - d)/2   at [2i,   2j+1]
  x10 = (a + v - h - d)/2   at [2i+1, 2j]
  x11 = (a - v - h + d)/2   at [2i+1, 2j+1]

Sharding: batch dim across 8 cores (1 batch each), no communication.

Per-core layout trick: view the (64,128,128) input as [128, 8192] where
partition P = 2*c + (i>=64) holds rows i in [64*(P%2), 64*(P%2)+64) of
channel c = P//2, each partition's data fully contiguous in DRAM. The
(64,256,256) output viewed as [128, 32768] has the *same* partition map
(P = 2*c + (i2>=128)), so input loads and output stores are both fully
contiguous DMAs with multi-KB descriptors.

Schedule (from ntff profiling): the SDMA engines sustain ~430 GB/s in
same-direction phases but only ~300-340 GB/s when loads and stores
interleave at packet granularity (HBM read/write turnaround). So all
DMA goes through ONE HWDGE ring (sync) whose FIFO order we craft into
multi-MiB same-direction bursts: 3 blocks of loads up front, then
alternating store-burst / load-burst. Only the post-last-load store
tail alternates rings (same direction, so mixing queues is free).

Compute: per 8-row unit, ACT prescale-casts avs=bf16([a|v]/2) and
hds=bf16([h|d]/2); DVE runs stage 1 ([p|q]=avs+hds, [r|s]=avs-hds) in
bf16 (2x perf mode) and stage 2 (butterfly TTs, bf16 in / fp32 out)
into the interleaved out tile. bf16 intermediates keep DVE well under
the DMA critical path; 3 bf16 roundings give ~0.6% worst-case error
vs the 2e-2 tolerance. GPSIMD stays idle: its SBUF traffic degrades
DVE throughput (measured 74us -> 89us) and Pool TT runs at ~0.26 eff.
"""

import numpy as np

B, C, H, W = 8, 64, 128, 128
N_CORES = 8
NP = 112  # SBUF partitions used: 7 full SDMA engine groups; E14/E15 idle
RMAX = 74  # row capacity per partition (16 x 74 + 96 x 73 = 8192)
BLOCKS = [8, 16, 16, 16, 18]  # row-chunks per pipeline block (sum RMAX)
PREFETCH = 3  # load blocks issued before the first store

_VALID = [74 if p < 16 else 73 for p in range(NP)]
_OFFS = np.concatenate([[0], np.cumsum(_VALID)]).astype(np.int64)

_cache = {}


def _build():
    import concourse.bacc as bacc
    import concourse.tile as tile
    from concourse import mybir

    fp32 = mybir.dt.float32
    bf16 = mybir.dt.bfloat16
    add = mybir.AluOpType.add
    sub = mybir.AluOpType.subtract

    nc = bacc.Bacc("TRN2", target_bir_lowering=False, debug=False)

    names = ["approximation", "detail_h", "detail_v", "detail_d"]
    ins = {
        n: nc.dram_tensor(n, [NP, RMAX * 128], fp32, kind="ExternalInput").ap()
        for n in names
    }
    out = nc.dram_tensor("out", [NP, RMAX * 512], fp32, kind="ExternalOutput").ap()

    with tile.TileContext(nc) as tc:
        with (
            tc.tile_pool(name="inp", bufs=PREFETCH) as inp,
            tc.tile_pool(name="avsp", bufs=2) as avsp,
            tc.tile_pool(name="hdsp", bufs=2) as hdsp,
            tc.tile_pool(name="pqp", bufs=2) as pqp,
            tc.tile_pool(name="outp", bufs=4) as outp,
        ):
            nb = len(BLOCKS)
            r0s = [sum(BLOCKS[:i]) for i in range(nb)]

            def emit_loads(i):
                rb = BLOCKS[i]
                FD = rb * 128
                isl = slice(r0s[i] * 128, (r0s[i] + rb) * 128)
                av = inp.tile([NP, 2 * FD], fp32, tag="av")
                hd = inp.tile([NP, 2 * FD], fp32, tag="hd")
                # first block: split across both HWDGE rings so the two
                # DGEs generate descriptors concurrently (no stores exist
                # yet, so no direction mixing); later loads stay on sync
                e2 = nc.scalar if i == 0 else nc.sync
                nc.sync.dma_start(out=av[:, 0:FD], in_=ins["approximation"][:, isl])
                e2.dma_start(out=av[:, FD : 2 * FD], in_=ins["detail_v"][:, isl])
                nc.sync.dma_start(out=hd[:, 0:FD], in_=ins["detail_h"][:, isl])
                e2.dma_start(out=hd[:, FD : 2 * FD], in_=ins["detail_d"][:, isl])
                return av, hd

            def emit_unit(av, hd, i, j, ur):
                # unit = rows [r0s[i]+j*8, +ur) of block i; ur <= 8
                rb = BLOCKS[i]
                HF = ur * 128
                c0 = j * 8 * 128
                av2 = av[:].rearrange("p (t f) -> p t f", t=2)[:, :, c0 : c0 + HF]
                hd2 = hd[:].rearrange("p (t f) -> p t f", t=2)[:, :, c0 : c0 + HF]

                avs = avsp.tile([NP, 2 * HF], bf16, tag="avs")
                avs2 = avs[:].rearrange("p (t f) -> p t f", t=2)
                nc.scalar.mul(avs2, av2, 0.5)  # bf16([a|v]/2)
                hds = hdsp.tile([NP, 2 * HF], bf16, tag="hds")
                hds2 = hds[:].rearrange("p (t f) -> p t f", t=2)
                nc.scalar.mul(hds2, hd2, 0.5)  # bf16([h|d]/2)

                pqrs = pqp.tile([NP, 4 * HF], bf16, tag="pqrs")
                pq = pqrs[:, 0 : 2 * HF].rearrange("p (t f) -> p t f", t=2)
                rs = pqrs[:, 2 * HF : 4 * HF].rearrange("p (t f) -> p t f", t=2)
                # [p|q] = [a|v]/2 + [h|d]/2 ; [r|s] = [a|v]/2 - [h|d]/2
                nc.vector.tensor_tensor(pq, avs2, hds2, add)
                nc.vector.tensor_tensor(rs, avs2, hds2, sub)

                to = outp.tile([NP, ur * 512], fp32, tag="o")
                v4 = pqrs[:].rearrange("p (t f) -> p t f", t=2)
                in0 = v4[:, :, 0:HF].rearrange("p t (r w) -> p t r w", w=128)
                in1 = v4[:, :, HF : 2 * HF].rearrange("p t (r w) -> p t r w", w=128)
                o4 = to[:].rearrange("p (r t x) -> p t r x", t=2, x=256)
                nc.vector.tensor_tensor(o4[:, :, :, 0:256:2], in0, in1, add)
                nc.vector.tensor_tensor(o4[:, :, :, 1:256:2], in0, in1, sub)
                return to

            def emit_store(to, i, j, ur, eng):
                w0 = (r0s[i] + j * 8) * 512
                eng.dma_start(out=out[:, w0 : w0 + ur * 512], in_=to[:])

            def units_of(i):
                rb = BLOCKS[i]
                if rb <= 8:
                    return [(0, rb)]
                return [(j, min(8, rb - j * 8)) for j in range((rb + 7) // 8)]

            tiles = {}
            for i in range(PREFETCH):
                tiles[i] = emit_loads(i)

            loads_emitted = PREFETCH
            store_eng = [nc.scalar, nc.sync]
            sidx = 0
            for i in range(nb):
                av, hd = tiles.pop(i)
                outs = [
                    (emit_unit(av, hd, i, j, ur), j, ur) for (j, ur) in units_of(i)
                ]
                if loads_emitted < nb:
                    # mixed phase: stores ride the sync ring in program
                    # order, forming same-direction bursts between loads
                    for to, j, ur in outs:
                        emit_store(to, i, j, ur, nc.sync)
                    tiles[loads_emitted] = emit_loads(loads_emitted)
                    loads_emitted += 1
                elif i < nb - 1:
                    # store-only tail on the sync ring (same direction)
                    for to, j, ur in outs:
                        emit_store(to, i, j, ur, nc.sync)
                else:
                    # final block's store on the idle scalar ring: drains
                    # concurrently with the sync ring's store backlog
                    for to, j, ur in outs:
                        emit_store(to, i, j, ur, nc.scalar)

    nc.compile()
    return nc


def _pack(v):
    # [64,128,128] f32 -> [NP, RMAX*128]: partition p = flat input rows
    # [_OFFS[p], _OFFS[p]+_VALID[p]), padded to RMAX rows with repeats
    rows = np.ascontiguousarray(v).reshape(64 * 128, 128)
    arr = np.empty((NP, RMAX * 128), dtype=np.float32)
    for p in range(NP):
        n = _VALID[p]
        chunk = rows[_OFFS[p] : _OFFS[p] + n].reshape(-1)
        arr[p, : n * 128] = chunk
        if n < RMAX:
            arr[p, n * 128 :] = chunk[: (RMAX - n) * 128]
    return arr


def _unpack(o):
    # [NP, RMAX*512] -> [64, 256, 256]: concat valid runs in flat-row order
    flat = np.concatenate([o[p, : _VALID[p] * 512] for p in range(NP)])
    return flat.reshape(C, 2 * H, 2 * W)


def kernel(approximation, detail_h, detail_v, detail_d):
    from concourse.bass_utils import run_bass_kernel_spmd

    if "nc" not in _cache:
        _cache["nc"] = _build()
    nc = _cache["nc"]

    full = {
        "approximation": approximation,
        "detail_h": detail_h,
        "detail_v": detail_v,
        "detail_d": detail_d,
    }
    in_maps = [
        {k: _pack(v[b]) for k, v in full.items()} for b in range(N_CORES)
    ]
    res = run_bass_kernel_spmd(nc, in_maps, list(range(N_CORES)))
    out = np.stack([_unpack(res.results[b]["out"]) for b in range(N_CORES)])
    return out.astype(np.float32, copy=False)


# revision 14
# speedup vs baseline: 1.0853x; 1.0853x over previous
"""Inverse 2D Haar DWT (idwt2) Trainium2 Bass kernel.

Full inputs: approximation/detail_h/detail_v/detail_d each [8, 64, 128, 128] f32.
Full output: [8, 64, 256, 256] f32 with out 2x2 blocks:
  x00 = (a + v + h + d)/2   at [2i,   2j]
  x01 = (a - v + h - d)/2   at [2i,   2j+1]
  x10 = (a + v - h - d)/2   at [2i+1, 2j]
  x11 = (a - v - h + d)/2   at [2i+1, 2j+1]

Sharding: batch dim across 8 cores (1 batch each), no communication.

Per-core layout trick: view the (64,128,128) input as [128, 8192] where
partition P = 2*c + (i>=64) holds rows i in [64*(P%2), 64*(P%2)+64) of
channel c = P//2, each partition's data fully contiguous in DRAM. The
(64,256,256) output viewed as [128, 32768] has the *same* partition map
(P = 2*c + (i2>=128)), so input loads and output stores are both fully
contiguous DMAs with multi-KB descriptors.

Schedule (from ntff profiling): the SDMA engines sustain ~430 GB/s in
same-direction phases but only ~300-340 GB/s when loads and stores
interleave at packet granularity (HBM read/write turnaround). So all
DMA goes through ONE HWDGE ring (sync) whose FIFO order we craft into
multi-MiB same-direction bursts: 3 blocks of loads up front, then
alternating store-burst / load-burst. Only the post-last-load store
tail alternates rings (same direction, so mixing queues is free).

Compute: per 8-row unit, ACT prescale-casts avs=bf16([a|v]/2) and
hds=bf16([h|d]/2); DVE runs stage 1 ([p|q]=avs+hds, [r|s]=avs-hds) in
bf16 (2x perf mode) and stage 2 (butterfly TTs, bf16 in / fp32 out)
into the interleaved out tile. bf16 intermediates keep DVE well under
the DMA critical path; 3 bf16 roundings give ~0.6% worst-case error
vs the 2e-2 tolerance. GPSIMD stays idle: its SBUF traffic degrades
DVE throughput (measured 74us -> 89us) and Pool TT runs at ~0.26 eff.
"""

import numpy as np

B, C, H, W = 8, 64, 128, 128
N_CORES = 8
BLOCKS = [8, 16, 16, 16, 8]  # rows per load block (sum 64)
PREFETCH = 3  # load blocks issued before the first store

_cache = {}


def _build():
    import concourse.bacc as bacc
    import concourse.tile as tile
    from concourse import mybir

    fp32 = mybir.dt.float32
    bf16 = mybir.dt.bfloat16
    add = mybir.AluOpType.add
    sub = mybir.AluOpType.subtract

    nc = bacc.Bacc("TRN2", target_bir_lowering=False, debug=False)

    names = ["approximation", "detail_h", "detail_v", "detail_d"]
    ins = {
        n: nc.dram_tensor(n, [128, 64 * 128], fp32, kind="ExternalInput").ap()
        for n in names
    }
    out = nc.dram_tensor("out", [128, 128 * 256], fp32, kind="ExternalOutput").ap()

    with tile.TileContext(nc) as tc:
        with (
            tc.tile_pool(name="inp", bufs=PREFETCH) as inp,
            tc.tile_pool(name="avsp", bufs=2) as avsp,
            tc.tile_pool(name="hdsp", bufs=2) as hdsp,
            tc.tile_pool(name="pqp", bufs=2) as pqp,
            tc.tile_pool(name="outp", bufs=4) as outp,
        ):
            nb = len(BLOCKS)
            r0s = [sum(BLOCKS[:i]) for i in range(nb)]

            def emit_loads(i):
                rb = BLOCKS[i]
                FD = rb * 128
                isl = slice(r0s[i] * 128, (r0s[i] + rb) * 128)
                av = inp.tile([128, 2 * FD], fp32, tag="av")
                hd = inp.tile([128, 2 * FD], fp32, tag="hd")
                # first block: split across both HWDGE rings so the two
                # DGEs generate descriptors concurrently (no stores exist
                # yet, so no direction mixing); later loads stay on sync
                e2 = nc.scalar if i == 0 else nc.sync
                nc.sync.dma_start(out=av[:, 0:FD], in_=ins["approximation"][:, isl])
                e2.dma_start(out=av[:, FD : 2 * FD], in_=ins["detail_v"][:, isl])
                nc.sync.dma_start(out=hd[:, 0:FD], in_=ins["detail_h"][:, isl])
                e2.dma_start(out=hd[:, FD : 2 * FD], in_=ins["detail_d"][:, isl])
                return av, hd

            def emit_unit(av, hd, i, j, ur):
                # unit = rows [r0s[i]+j*8, +ur) of block i; ur <= 8
                rb = BLOCKS[i]
                HF = ur * 128
                c0 = j * 8 * 128
                av2 = av[:].rearrange("p (t f) -> p t f", t=2)[:, :, c0 : c0 + HF]
                hd2 = hd[:].rearrange("p (t f) -> p t f", t=2)[:, :, c0 : c0 + HF]

                avs = avsp.tile([128, 2 * HF], bf16, tag="avs")
                avs2 = avs[:].rearrange("p (t f) -> p t f", t=2)
                nc.scalar.mul(avs2, av2, 0.5)  # bf16([a|v]/2)
                hds = hdsp.tile([128, 2 * HF], bf16, tag="hds")
                hds2 = hds[:].rearrange("p (t f) -> p t f", t=2)
                nc.scalar.mul(hds2, hd2, 0.5)  # bf16([h|d]/2)

                pqrs = pqp.tile([128, 4 * HF], bf16, tag="pqrs")
                pq = pqrs[:, 0 : 2 * HF].rearrange("p (t f) -> p t f", t=2)
                rs = pqrs[:, 2 * HF : 4 * HF].rearrange("p (t f) -> p t f", t=2)
                # [p|q] = [a|v]/2 + [h|d]/2 ; [r|s] = [a|v]/2 - [h|d]/2
                nc.vector.tensor_tensor(pq, avs2, hds2, add)
                nc.vector.tensor_tensor(rs, avs2, hds2, sub)

                to = outp.tile([128, ur * 512], fp32, tag="o")
                v4 = pqrs[:].rearrange("p (t f) -> p t f", t=2)
                in0 = v4[:, :, 0:HF].rearrange("p t (r w) -> p t r w", w=128)
                in1 = v4[:, :, HF : 2 * HF].rearrange("p t (r w) -> p t r w", w=128)
                o4 = to[:].rearrange("p (r t x) -> p t r x", t=2, x=256)
                nc.vector.tensor_tensor(o4[:, :, :, 0:256:2], in0, in1, add)
                nc.vector.tensor_tensor(o4[:, :, :, 1:256:2], in0, in1, sub)
                return to

            def emit_store(to, i, j, ur, eng):
                w0 = (r0s[i] + j * 8) * 512
                eng.dma_start(out=out[:, w0 : w0 + ur * 512], in_=to[:])

            def units_of(i):
                rb = BLOCKS[i]
                if rb <= 8:
                    return [(0, rb)]
                return [(j, min(8, rb - j * 8)) for j in range((rb + 7) // 8)]

            tiles = {}
            for i in range(PREFETCH):
                tiles[i] = emit_loads(i)

            loads_emitted = PREFETCH
            store_eng = [nc.scalar, nc.sync]
            sidx = 0
            for i in range(nb):
                av, hd = tiles.pop(i)
                outs = [
                    (emit_unit(av, hd, i, j, ur), j, ur) for (j, ur) in units_of(i)
                ]
                if loads_emitted < nb:
                    # mixed phase: stores ride the sync ring in program
                    # order, forming same-direction bursts between loads
                    for to, j, ur in outs:
                        emit_store(to, i, j, ur, nc.sync)
                    tiles[loads_emitted] = emit_loads(loads_emitted)
                    loads_emitted += 1
                elif i < nb - 1:
                    # store-only tail on the sync ring (same direction)
                    for to, j, ur in outs:
                        emit_store(to, i, j, ur, nc.sync)
                else:
                    # final block's store on the idle scalar ring: drains
                    # concurrently with the sync ring's store backlog
                    for to, j, ur in outs:
                        emit_store(to, i, j, ur, nc.scalar)

    nc.compile()
    return nc


def kernel(approximation, detail_h, detail_v, detail_d):
    from concourse.bass_utils import run_bass_kernel_spmd

    if "nc" not in _cache:
        _cache["nc"] = _build()
    nc = _cache["nc"]

    full = {
        "approximation": approximation,
        "detail_h": detail_h,
        "detail_v": detail_v,
        "detail_d": detail_d,
    }
    in_maps = [
        {
            k: np.ascontiguousarray(v[b]).reshape(128, 64 * 128)
            for k, v in full.items()
        }
        for b in range(N_CORES)
    ]
    res = run_bass_kernel_spmd(nc, in_maps, list(range(N_CORES)))
    out = np.stack(
        [res.results[b]["out"].reshape(C, 2 * H, 2 * W) for b in range(N_CORES)]
    )
    return out.astype(np.float32, copy=False)


# revision 15
# speedup vs baseline: 1.4602x; 1.3454x over previous
"""Inverse 2D Haar DWT (idwt2) Trainium2 Bass kernel.

Full inputs: approximation/detail_h/detail_v/detail_d each [8, 64, 128, 128] f32.
Full output: [8, 64, 256, 256] f32 with out 2x2 blocks:
  x00 = (a + v + h + d)/2   at [2i,   2j]
  x01 = (a - v + h - d)/2   at [2i,   2j+1]
  x10 = (a + v - h - d)/2   at [2i+1, 2j]
  x11 = (a - v - h + d)/2   at [2i+1, 2j+1]

Sharding: batch dim across 8 cores (1 batch each), no communication.

Per-core layout trick: view the (64,128,128) input as [128, 8192] where
partition P = 2*c + (i>=64) holds rows i in [64*(P%2), 64*(P%2)+64) of
channel c = P//2, each partition's data fully contiguous in DRAM. The
(64,256,256) output viewed as [128, 32768] has the *same* partition map
(P = 2*c + (i2>=128)), so input loads and output stores are both fully
contiguous DMAs with multi-KB descriptors.

Schedule (from ntff profiling): the SDMA engines sustain ~430 GB/s in
same-direction phases but only ~300-340 GB/s when loads and stores
interleave at packet granularity (HBM read/write turnaround). So all
DMA goes through ONE HWDGE ring (sync) whose FIFO order we craft into
multi-MiB same-direction bursts: 3 blocks of loads up front, then
alternating store-burst / load-burst. Only the post-last-load store
tail alternates rings (same direction, so mixing queues is free).

Compute: per 8-row unit, ACT prescale-casts avs=bf16([a|v]/2) and
hds=bf16([h|d]/2); DVE runs stage 1 ([p|q]=avs+hds, [r|s]=avs-hds) in
bf16 (2x perf mode) and stage 2 (butterfly TTs, bf16 in / fp32 out)
into the interleaved out tile. bf16 intermediates keep DVE well under
the DMA critical path; 3 bf16 roundings give ~0.6% worst-case error
vs the 2e-2 tolerance. GPSIMD stays idle: its SBUF traffic degrades
DVE throughput (measured 74us -> 89us) and Pool TT runs at ~0.26 eff.
"""

import numpy as np

B, C, H, W = 8, 64, 128, 128
N_CORES = 8
BLOCKS = [8, 16, 16, 16, 8]  # rows per load block (sum 64)
PREFETCH = 3  # load blocks issued before the first store

_cache = {}


def _build():
    import concourse.bacc as bacc
    import concourse.tile as tile
    from concourse import mybir

    fp32 = mybir.dt.float32
    bf16 = mybir.dt.bfloat16
    add = mybir.AluOpType.add
    sub = mybir.AluOpType.subtract

    nc = bacc.Bacc("TRN2", target_bir_lowering=False, debug=False)

    names = ["approximation", "detail_h", "detail_v", "detail_d"]
    ins = {
        n: nc.dram_tensor(n, [128, 64 * 128], fp32, kind="ExternalInput").ap()
        for n in names
    }
    # output in bf16 (halves store traffic; upconverted to fp32 on host —
    # one extra rounding inside the 2e-2 tolerance)
    out = nc.dram_tensor("out", [128, 128 * 256], bf16, kind="ExternalOutput").ap()

    with tile.TileContext(nc) as tc:
        with (
            tc.tile_pool(name="inp", bufs=PREFETCH) as inp,
            tc.tile_pool(name="avsp", bufs=2) as avsp,
            tc.tile_pool(name="hdsp", bufs=2) as hdsp,
            tc.tile_pool(name="pqp", bufs=2) as pqp,
            tc.tile_pool(name="outp", bufs=4) as outp,
        ):
            nb = len(BLOCKS)
            r0s = [sum(BLOCKS[:i]) for i in range(nb)]

            def emit_loads(i):
                rb = BLOCKS[i]
                FD = rb * 128
                isl = slice(r0s[i] * 128, (r0s[i] + rb) * 128)
                av = inp.tile([128, 2 * FD], fp32, tag="av")
                hd = inp.tile([128, 2 * FD], fp32, tag="hd")
                # first block: split across both HWDGE rings so the two
                # DGEs generate descriptors concurrently (no stores exist
                # yet, so no direction mixing); later loads stay on sync
                e2 = nc.scalar if i == 0 else nc.sync
                nc.sync.dma_start(out=av[:, 0:FD], in_=ins["approximation"][:, isl])
                e2.dma_start(out=av[:, FD : 2 * FD], in_=ins["detail_v"][:, isl])
                nc.sync.dma_start(out=hd[:, 0:FD], in_=ins["detail_h"][:, isl])
                e2.dma_start(out=hd[:, FD : 2 * FD], in_=ins["detail_d"][:, isl])
                return av, hd

            def emit_unit(av, hd, i, j, ur):
                # unit = rows [r0s[i]+j*8, +ur) of block i; ur <= 8
                rb = BLOCKS[i]
                HF = ur * 128
                c0 = j * 8 * 128
                av2 = av[:].rearrange("p (t f) -> p t f", t=2)[:, :, c0 : c0 + HF]
                hd2 = hd[:].rearrange("p (t f) -> p t f", t=2)[:, :, c0 : c0 + HF]

                avs = avsp.tile([128, 2 * HF], bf16, tag="avs")
                avs2 = avs[:].rearrange("p (t f) -> p t f", t=2)
                nc.scalar.mul(avs2, av2, 0.5)  # bf16([a|v]/2)
                hds = hdsp.tile([128, 2 * HF], bf16, tag="hds")
                hds2 = hds[:].rearrange("p (t f) -> p t f", t=2)
                nc.scalar.mul(hds2, hd2, 0.5)  # bf16([h|d]/2)

                pqrs = pqp.tile([128, 4 * HF], bf16, tag="pqrs")
                pq = pqrs[:, 0 : 2 * HF].rearrange("p (t f) -> p t f", t=2)
                rs = pqrs[:, 2 * HF : 4 * HF].rearrange("p (t f) -> p t f", t=2)
                # [p|q] = [a|v]/2 + [h|d]/2 ; [r|s] = [a|v]/2 - [h|d]/2
                nc.vector.tensor_tensor(pq, avs2, hds2, add)
                nc.vector.tensor_tensor(rs, avs2, hds2, sub)

                to = outp.tile([128, ur * 512], bf16, tag="o")
                v4 = pqrs[:].rearrange("p (t f) -> p t f", t=2)
                in0 = v4[:, :, 0:HF].rearrange("p t (r w) -> p t r w", w=128)
                in1 = v4[:, :, HF : 2 * HF].rearrange("p t (r w) -> p t r w", w=128)
                o4 = to[:].rearrange("p (r t x) -> p t r x", t=2, x=256)
                nc.vector.tensor_tensor(o4[:, :, :, 0:256:2], in0, in1, add)
                nc.vector.tensor_tensor(o4[:, :, :, 1:256:2], in0, in1, sub)
                return to

            def emit_store(to, i, j, ur, eng):
                w0 = (r0s[i] + j * 8) * 512
                eng.dma_start(out=out[:, w0 : w0 + ur * 512], in_=to[:])

            def units_of(i):
                rb = BLOCKS[i]
                if rb <= 8:
                    return [(0, rb)]
                return [(j, min(8, rb - j * 8)) for j in range((rb + 7) // 8)]

            tiles = {}
            for i in range(PREFETCH):
                tiles[i] = emit_loads(i)

            loads_emitted = PREFETCH
            store_eng = [nc.scalar, nc.sync]
            sidx = 0
            for i in range(nb):
                av, hd = tiles.pop(i)
                outs = [
                    (emit_unit(av, hd, i, j, ur), j, ur) for (j, ur) in units_of(i)
                ]
                if loads_emitted < nb:
                    # mixed phase: stores ride the sync ring in program
                    # order, forming same-direction bursts between loads
                    for to, j, ur in outs:
                        emit_store(to, i, j, ur, nc.sync)
                    tiles[loads_emitted] = emit_loads(loads_emitted)
                    loads_emitted += 1
                elif i < nb - 1:
                    # store-only tail on the sync ring (same direction)
                    for to, j, ur in outs:
                        emit_store(to, i, j, ur, nc.sync)
                else:
                    # final block's store on the idle scalar ring: drains
                    # concurrently with the sync ring's store backlog
                    for to, j, ur in outs:
                        emit_store(to, i, j, ur, nc.scalar)

    nc.compile()
    return nc


def kernel(approximation, detail_h, detail_v, detail_d):
    from concourse.bass_utils import run_bass_kernel_spmd

    if "nc" not in _cache:
        _cache["nc"] = _build()
    nc = _cache["nc"]

    full = {
        "approximation": approximation,
        "detail_h": detail_h,
        "detail_v": detail_v,
        "detail_d": detail_d,
    }
    in_maps = [
        {
            k: np.ascontiguousarray(v[b]).reshape(128, 64 * 128)
            for k, v in full.items()
        }
        for b in range(N_CORES)
    ]
    res = run_bass_kernel_spmd(nc, in_maps, list(range(N_CORES)))
    out = np.stack(
        [
            res.results[b]["out"].astype(np.float32).reshape(C, 2 * H, 2 * W)
            for b in range(N_CORES)
        ]
    )
    return out
